# revision 3
# baseline (speedup 1.0000x reference)
"""BitLinear (RMSNorm + int8-absmax activation quant + ternary weight quant + matmul)
on 8 Trainium2 NeuronCores.

Strategy:
  - Shard rows of x across cores (256 rows each): RMSNorm + local absmax.
  - Shard weight columns across cores ([4096, 512] each): local sum(|W|).
  - AllGather the two scalars -> global a_scale / b_scale (exact semantics).
  - Quantize activations to bf16 ints in [-127, 127] (exact in bf16),
    AllGather the quantized activation matrix (bf16, 16.8 MB total).
  - Quantize local weight shard to ternary bf16.
  - Matmul A_q @ B_t per core: lhsT tiles come from hardware DMA-transpose
    loads of the gathered bf16 activations; accumulate K=4096 in PSUM over
    32 k-tiles; dequant fused into the PSUM->SBUF copy.
  - Each core writes its [2048, 512] output column shard; host concatenates.

Self-contained: only needs numpy + the platform's concourse/bass libraries.
"""

import os
import sys

import numpy as np

for _p in ("/opt/trn_rl_repo", "/root/.axon_site/_ro/trn_rl_repo"):
    if os.path.isdir(_p) and _p not in sys.path:
        sys.path.append(_p)

import concourse.bass as bass
import concourse.tile as tile
from concourse import mybir
from concourse.bass_utils import run_bass_kernel_spmd

R = 8  # cores
M, K, N = 2048, 4096, 4096
M_LOC = M // R  # 256 rows of x per core
N_LOC = N // R  # 512 weight columns per core
P = 128
KT = K // P  # 32 k-tiles
MT_LOC = M_LOC // P  # 2 m-tiles per core
EPS_RMS = 1e-6
Q_CLIP = 1e-5
MAGIC = 12582912.0  # 1.5 * 2**23: (v + MAGIC) - MAGIC == round-to-nearest-even(v)
F32 = mybir.dt.float32
BF16 = mybir.dt.bfloat16
AX = mybir.AxisListType
ALU = mybir.AluOpType


def _split_waits(nc, max_waits=1):
    """This toolchain rejects instructions with several semaphore waits
    ("Too many sync wait commands"). Hoist excess waits onto no-op
    instructions just before the offender on the same engine."""
    counter = 0
    for f in nc.m.functions:
        for blk in f.blocks:
            new_insts = []
            for inst in blk.instructions:
                si = getattr(inst, "sync_info", None)
                waits = list(si.on_wait) if si is not None and si.on_wait else []
                if len(waits) > max_waits:
                    excess = waits[: len(waits) - max_waits]
                    keep = waits[len(waits) - max_waits :]
                    for i in range(0, len(excess), max_waits):
                        counter += 1
                        nop = mybir.InstNoOp(
                            name=f"waitsplit_{counter}_{inst.name}", ins=[], outs=[]
                        )
                        nop.engine = inst.engine
                        nop.bass_nofuse = True
                        nop.sync_info = mybir.SyncInfo(
                            on_wait=list(excess[i : i + max_waits]), on_update=[]
                        )
                        new_insts.append(nop)
                    si.on_wait = keep
                    inst.sync_info = si
                new_insts.append(inst)
            blk.instructions[:] = new_insts


def _bcast_ap(ap, p):
    """Broadcast a 1-D DRAM AP across p partitions (step-0 partition axis)."""
    return bass.AP(tensor=ap.tensor, offset=ap.offset, ap=[[0, p]] + list(ap.ap))


def build_kernel():
    nc = bass.Bass(num_devices=R)
    rg = [list(range(R))]

    x_in = nc.declare_dram_parameter("x_loc", [M_LOC, K], F32, isOutput=False)
    w_in = nc.declare_dram_parameter("w_loc", [K, N_LOC], F32, isOutput=False)
    rms_in = nc.declare_dram_parameter("rms_w", [K], F32, isOutput=False)
    out_ext = nc.declare_dram_parameter("out_loc", [M, N_LOC], F32, isOutput=True)

    stats_loc = nc.dram_tensor("stats_loc", [2], F32)
    stats_all = nc.dram_tensor("stats_all", [2 * R], F32, addr_space="Shared")
    scal_dram = nc.dram_tensor("scal_dram", [3], F32)
    aq_loc = nc.dram_tensor("aq_loc", [M_LOC, K], BF16)
    aq_all = nc.dram_tensor("aq_all", [M, K], BF16, addr_space="Shared")

    with tile.TileContext(nc) as tc:
        ctxs = [
            tc.tile_pool(name="wres", bufs=1),
            tc.tile_pool(name="btres", bufs=1),
            tc.tile_pool(name="rmsp", bufs=1),
            tc.tile_pool(name="xz", bufs=2),
            tc.tile_pool(name="aq", bufs=2),
            tc.tile_pool(name="st", bufs=2),
            tc.tile_pool(name="lhs", bufs=3),
            tc.tile_pool(name="psum", bufs=8, space="PSUM"),
            tc.tile_pool(name="outp", bufs=4),
            tc.tile_pool(name="small", bufs=1),
        ]
        from contextlib import ExitStack

        with ExitStack() as es:
            (wres_p, bt_p, rms_p, xz_p, aq_p, st_p, lhs_p, psum_p, out_p, small_p) = [
                es.enter_context(c) for c in ctxs
            ]

            eps_t = small_p.tile([P, 1], F32)
            nc.vector.memset(eps_t, EPS_RMS)

            # ---------- W shard: load resident + abs-sum stats ----------
            w_res = wres_p.tile([P, KT, N_LOC], F32)
            nc.sync.dma_start(
                w_res[:], w_in[:, :].rearrange("(kt p) n -> p kt n", p=P)
            )
            wsum32 = small_p.tile([P, KT], F32)
            nc.vector.tensor_reduce(
                out=wsum32,
                in_=w_res[:],
                axis=AX.X,
                op=ALU.add,
                apply_absolute_value=True,
            )
            wsum_pp = small_p.tile([P, 1], F32)
            nc.vector.tensor_reduce(out=wsum_pp, in_=wsum32, axis=AX.X, op=ALU.add)
            wsum_s = small_p.tile([1, 1], F32)
            nc.gpsimd.tensor_reduce(out=wsum_s, in_=wsum_pp, axis=AX.C, op=ALU.add)

            # ---------- rms_weight broadcast ----------
            rms_b = rms_p.tile([P, K], F32)
            nc.sync.dma_start(rms_b[:], _bcast_ap(rms_in[:], P))

            # ---------- x rows: RMS norm + local absmax ----------
            amax_mt = small_p.tile([P, MT_LOC], F32)
            z_tiles = []
            for mt in range(MT_LOC):
                xz = xz_p.tile([P, K], F32)
                nc.sync.dma_start(xz[:], x_in[mt * P : (mt + 1) * P, :])
                xg = xz[:].rearrange("p (g d) -> p g d", d=512)
                stats6 = st_p.tile([P, 8, 6], F32)
                for g in range(8):
                    nc.vector.bn_stats(out=stats6[:, g, :], in_=xg[:, g, :])
                mv = st_p.tile([P, 2], F32)
                nc.vector.bn_aggr(out=mv, in_=stats6[:])
                # mean(x^2) = var + mean^2
                msq = st_p.tile([P, 1], F32)
                nc.vector.tensor_tensor(
                    out=msq, in0=mv[:, 0:1], in1=mv[:, 0:1], op=ALU.mult
                )
                nc.vector.tensor_tensor(out=msq, in0=msq, in1=mv[:, 1:2], op=ALU.add)
                # r = 1/sqrt(msq + eps)
                r_t = st_p.tile([P, 1], F32)
                nc.scalar.activation(
                    out=r_t,
                    in_=msq,
                    func=mybir.ActivationFunctionType.Sqrt,
                    bias=eps_t,
                    scale=1.0,
                )
                nc.vector.reciprocal(out=r_t, in_=r_t)
                # z = (x * r) * rms_weight   (in place)
                nc.vector.tensor_scalar_mul(out=xz[:], in0=xz[:], scalar1=r_t)
                nc.vector.tensor_tensor(
                    out=xz[:], in0=xz[:], in1=rms_b[:], op=ALU.mult
                )
                nc.vector.tensor_reduce(
                    out=amax_mt[:, mt : mt + 1],
                    in_=xz[:],
                    axis=AX.X,
                    op=ALU.max,
                    apply_absolute_value=True,
                )
                z_tiles.append(xz)

            amax_pp = small_p.tile([P, 1], F32)
            nc.vector.tensor_reduce(out=amax_pp, in_=amax_mt[:], axis=AX.X, op=ALU.max)
            amax_s = small_p.tile([1, 1], F32)
            nc.gpsimd.tensor_reduce(out=amax_s, in_=amax_pp, axis=AX.C, op=ALU.max)

            # ---------- stats allgather -> global scales ----------
            stats_sb = small_p.tile([1, 2], F32)
            nc.vector.tensor_copy(out=stats_sb[:, 0:1], in_=amax_s[:])
            nc.vector.tensor_copy(out=stats_sb[:, 1:2], in_=wsum_s[:])
            nc.sync.dma_start(stats_loc[None, :], stats_sb[0:1, :])
            nc.gpsimd.collective_compute(
                "AllGather",
                ALU.bypass,
                replica_groups=rg,
                ins=[stats_loc[:]],
                outs=[stats_all[:]],
            )
            sb16 = small_p.tile([1, 2 * R], F32)
            nc.sync.dma_start(sb16[0:1, :], stats_all[None, :])
            v = sb16[:].rearrange("p (r t) -> p r t", t=2)
            gmax = small_p.tile([1, 1], F32)
            gsum = small_p.tile([1, 1], F32)
            nc.vector.tensor_reduce(out=gmax, in_=v[:, :, 0:1], axis=AX.XY, op=ALU.max)
            nc.vector.tensor_reduce(out=gsum, in_=v[:, :, 1:2], axis=AX.XY, op=ALU.add)
            # gmax <- clip(absmax); gsum <- clip(mean|W|)
            nc.vector.tensor_scalar_max(out=gmax, in0=gmax, scalar1=Q_CLIP)
            nc.vector.tensor_scalar(
                out=gsum,
                in0=gsum,
                scalar1=1.0 / (K * N),
                scalar2=Q_CLIP,
                op0=ALU.mult,
                op1=ALU.max,
            )
            a_s = small_p.tile([1, 1], F32)
            nc.vector.reciprocal(out=a_s, in_=gmax)
            nc.vector.tensor_scalar_mul(out=a_s, in0=a_s, scalar1=127.0)
            b_s = small_p.tile([1, 1], F32)
            nc.vector.reciprocal(out=b_s, in_=gsum)
            dq = small_p.tile([1, 1], F32)
            nc.vector.tensor_tensor(out=dq, in0=gmax, in1=gsum, op=ALU.mult)
            nc.vector.tensor_scalar_mul(out=dq, in0=dq, scalar1=1.0 / 127.0)
            scal_sb = small_p.tile([1, 3], F32)
            nc.vector.tensor_copy(out=scal_sb[:, 0:1], in_=a_s[:])
            nc.vector.tensor_copy(out=scal_sb[:, 1:2], in_=b_s[:])
            nc.vector.tensor_copy(out=scal_sb[:, 2:3], in_=dq[:])
            nc.sync.dma_start(scal_dram[None, :], scal_sb[0:1, :])
            scal_b = small_p.tile([P, 3], F32)
            nc.sync.dma_start(scal_b[:], _bcast_ap(scal_dram[:], P))

            # ---------- activation quant (bf16 ints) + allgather ----------
            for mt in range(MT_LOC):
                z = z_tiles[mt]
                # z <- z * a_scale + MAGIC ; aq <- z - MAGIC  (round to nearest)
                nc.vector.tensor_scalar(
                    out=z[:],
                    in0=z[:],
                    scalar1=scal_b[:, 0:1],
                    scalar2=MAGIC,
                    op0=ALU.mult,
                    op1=ALU.add,
                )
                aq_t = aq_p.tile([P, K], BF16)
                nc.vector.tensor_scalar(
                    out=aq_t[:], in0=z[:], scalar1=MAGIC, scalar2=None, op0=ALU.subtract
                )
                nc.gpsimd.dma_start(aq_loc[mt * P : (mt + 1) * P, :], aq_t[:])
            nc.gpsimd.collective_compute(
                "AllGather",
                ALU.bypass,
                replica_groups=rg,
                ins=[aq_loc[:, :]],
                outs=[aq_all[:, :]],
            )

            # ---------- weight quant: ternary bf16 ----------
            bt = bt_p.tile([P, KT, N_LOC], BF16)
            nc.vector.tensor_scalar(
                out=w_res[:],
                in0=w_res[:],
                scalar1=scal_b[:, 1:2],
                scalar2=MAGIC,
                op0=ALU.mult,
                op1=ALU.add,
            )
            nc.vector.tensor_scalar(
                out=w_res[:],
                in0=w_res[:],
                scalar1=MAGIC,
                scalar2=1.0,
                op0=ALU.subtract,
                op1=ALU.min,
            )
            nc.vector.tensor_scalar(
                out=bt[:], in0=w_res[:], scalar1=-1.0, scalar2=None, op0=ALU.max
            )

            # ---------- matmul: out[m, n_loc] = A_q @ B_t, dequant fused ----------
            HALF_MT = 8  # 8 m-tiles per half => 8 PSUM banks
            for half in range(2):
                psums = [
                    psum_p.tile([P, N_LOC], F32, tag="ps", name=f"ps_{half}_{i}")
                    for i in range(HALF_MT)
                ]
                for kt in range(KT):
                    lhsT = lhs_p.tile([P, HALF_MT * P], BF16)
                    nc.sync.dma_start_transpose(
                        lhsT[:],
                        aq_all[
                            half * HALF_MT * P : (half + 1) * HALF_MT * P,
                            kt * P : (kt + 1) * P,
                        ],
                    )
                    for mt in range(HALF_MT):
                        nc.tensor.matmul(
                            psums[mt][:],
                            lhsT[:, mt * P : (mt + 1) * P],
                            bt[:, kt, :],
                            start=(kt == 0),
                            stop=(kt == KT - 1),
                        )
                for mt in range(HALF_MT):
                    o_t = out_p.tile([P, N_LOC], F32)
                    nc.vector.tensor_scalar_mul(
                        out=o_t[:], in0=psums[mt][:], scalar1=scal_b[:, 2:3]
                    )
                    gm = half * HALF_MT + mt
                    nc.gpsimd.dma_start(out_ext[gm * P : (gm + 1) * P, :], o_t[:])

    _split_waits(nc)
    return nc


_CACHE = {}


def _get_nc():
    if "nc" not in _CACHE:
        _CACHE["nc"] = build_kernel()
    return _CACHE["nc"]


def make_in_maps(x, weight, rms_weight):
    x = np.ascontiguousarray(np.asarray(x, dtype=np.float32)).reshape(M, K)
    weight = np.asarray(weight, dtype=np.float32)
    rms_weight = np.ascontiguousarray(np.asarray(rms_weight, dtype=np.float32))
    return [
        {
            "x_loc": np.ascontiguousarray(x[c * M_LOC : (c + 1) * M_LOC]),
            "w_loc": np.ascontiguousarray(weight[:, c * N_LOC : (c + 1) * N_LOC]),
            "rms_w": rms_weight,
        }
        for c in range(R)
    ]


def assemble_out(results):
    out = np.concatenate([results[c]["out_loc"] for c in range(R)], axis=1)
    return out.reshape(1, M, N)


def kernel(x, weight, rms_weight):
    nc = _get_nc()
    in_maps = make_in_maps(x, weight, rms_weight)
    res = run_bass_kernel_spmd(nc, in_maps, core_ids=list(range(R)))
    return assemble_out(res.results)
